# revision 10
# baseline (speedup 1.0000x reference)
"""MinRNN Trainium2 Bass kernel.

Problem: minLSTM-style recurrence over sentences.
  x = emb[sentence]                       [B,S,E]
  f = sigmoid(x@Wf + bf); i = sigmoid(x@Wi + bi); h~ = x@Wh + bh
  f_n = f/(f+i); g = (i/(f+i)) * h~
  h_t = f_n_t * h_{t-1} + g_t   (scan over S, only final h needed)
  out = sigmoid((h@W1 + b1)@W2 + b2)      [B,1]

Sharding: data-parallel over batch. 8 cores x 8 rows each. Weights and the
embedding table are replicated; each core gathers its own tokens from the
table with indirect DMA.

Per-core dataflow (all shapes per core; ROWS=8, S=1024, E=U=1024):
  - gather 128-token tiles of emb rows -> SBUF [128 tok, E] f32
  - cast to bf16 (ScalarE), DMA-transpose to xT [128 e, EB, tok] bf16
  - 3 GEMMs on TensorE in bf16 (fp32 PSUM accumulate), N=512 moving dim
  - gate math on ScalarE/VectorE in fp32
  - tensor_tensor_scan on VectorE for the recurrence (fp32 state)
  - tiny head matmuls in fp32, sigmoid, DMA out [1, ROWS]
"""

import sys

if "/opt/trn_rl_repo" not in sys.path:
    sys.path.insert(0, "/opt/trn_rl_repo")

import numpy as np
import ml_dtypes

import concourse.bass as bass
import concourse.bacc as bacc
import concourse.mybir as mybir
from concourse.bass import ts
from concourse.tile import TileContext
from concourse.bass_utils import run_bass_kernel_spmd

N_CORES = 8
B, S, E, U, V = 64, 1024, 1024, 1024, 32000

F32 = mybir.dt.float32
BF16 = mybir.dt.bfloat16
I32 = mybir.dt.int32
AF = mybir.ActivationFunctionType
ALU = mybir.AluOpType


def _register_dve_op(name, spec):
    """Register a custom DVE op at runtime (self-pinning its uops sha)."""
    from concourse import dve_ops
    from concourse.dve_spec import lower, _has_src1
    from concourse.dve_uop import DveOpSpec

    if name in dve_ops.CUSTOM_DVE_SPECS:
        for op in dve_ops.OPS:
            if op.name == name:
                return op
    dve_ops._SUB_OPCODE_FOR_NAME[name] = dve_ops._CUSTOM_DVE_ROW_BASE + len(
        dve_ops.OPS
    )
    shas = {}
    for ver in ("v3", "v4"):
        s = DveOpSpec(
            name=name,
            opcode=dve_ops.get_dve_sub_opcode(name),
            uops=lower(spec, ver=ver),
            rd1_en=_has_src1(spec),
        )
        shas[ver] = s.sha(ver)
    op = dve_ops.DveOp(name, spec, subdim=False, uops_sha=shas)
    dve_ops.OPS.append(op)
    dve_ops.CUSTOM_DVE_SPECS[name] = spec
    return op


def _make_gate_ops():
    """Two fused gate ops:

    MINRNN_FN: fn = f / (f + i) via BITWISE_NOT reciprocal seed + 1 Newton
      step (Chebyshev pair; ~1.7e-3 max rel err on den in (0,2)).
      in0=f, in1=i, s0/s1 = recip constants.
    MINRNN_GG: gg = (h_pre + bh) * (1 - fn).  in0=h_pre(psum), in1=fn, s0=bh.
    """
    import numpy as np
    from concourse.dve_spec import AluOp, Bin, C0, C1, One, Spec, Src0, Src1

    _den = Src0 + Src1
    _nd = Bin(AluOp.BITWISE_NOT, _den, _den)
    _y0 = _nd * C0
    _y1 = _y0 * (C1 - _den * _y0)

    def _ref_fn(in0, in1, c0, c1, c2):
        den = (in0 + in1).astype(np.float32)
        nd = (~den.view(np.int32)).view(np.float32)
        y0 = (nd * np.float32(c0)).astype(np.float32)
        y1 = (y0 * (np.float32(c1) - den * y0)).astype(np.float32)
        return (in0 * y1).astype(np.float32)

    fn_op = _register_dve_op(
        "MINRNN_FN", Spec(body=Src0 * _y1, reference=_ref_fn)
    )

    def _ref_gg(in0, in1, c0, c1, c2):
        c0 = np.asarray(c0, np.float32)
        return ((in0 + c0) * (np.float32(1.0) - in1)).astype(np.float32)

    gg_op = _register_dve_op(
        "MINRNN_GG",
        Spec(body=(Src0 + C0) * (One - Src1), reference=_ref_gg),
    )
    return fn_op, gg_op


RECIP_C0 = -0.23549792
RECIP_C1 = 2.0017324


def build_nc(n_rows=B // N_CORES, s=S, e=E, u=U, v=V, tok_tile=512):
    """Build the single-core program (SPMD: same program on all cores)."""
    assert s % tok_tile == 0 and tok_tile % 128 == 0
    toks = n_rows * s            # tokens per core
    G = toks // 128              # number of 128-row gathers
    NT = toks // tok_tile        # token tiles
    QT = tok_tile // 128         # gathers per token tile
    EB = e // 128                # contraction blocks
    UB = u // 128                # output-unit blocks
    tt_per_row = s // tok_tile

    nc = bacc.Bacc("TRN2", target_bir_lowering=False)
    FN_OP, GG_OP = _make_gate_ops()

    idx_t = nc.dram_tensor("idx", [128, G], I32, kind="ExternalInput")
    emb_t = nc.dram_tensor("emb", [v, e], F32, kind="ExternalInput")
    w_t = {
        n: nc.dram_tensor(n, [128, EB, u], BF16, kind="ExternalInput")
        for n in ("wf", "wi", "wh")
    }
    b_t = {
        n: nc.dram_tensor(n, [128, UB], F32, kind="ExternalInput")
        for n in ("bfv", "biv", "bhv")
    }
    w1_t = nc.dram_tensor("w1", [128, UB, 64], F32, kind="ExternalInput")
    w2_t = nc.dram_tensor("w2", [64, 1], F32, kind="ExternalInput")
    b1_t = nc.dram_tensor("b1", [64, 1], F32, kind="ExternalInput")
    b2_t = nc.dram_tensor("b2", [1, 1], F32, kind="ExternalInput")
    out_t = nc.dram_tensor("out", [1, n_rows], F32, kind="ExternalOutput")

    with TileContext(nc) as tc:
        with (
            tc.tile_pool(name="singles", bufs=1) as singles,
            tc.tile_pool(name="xraw", bufs=4) as xraw_p,
            tc.tile_pool(name="xbf", bufs=4) as xbf_p,
            tc.tile_pool(name="xT", bufs=3) as xT_p,
            tc.tile_pool(name="sig", bufs=4) as sig_p,
            tc.tile_pool(name="gw", bufs=3) as gw_p,
            tc.tile_pool(name="scan", bufs=2 * UB) as scan_p,
            tc.tile_pool(name="gates", bufs=6, space="PSUM") as gps_p,
            tc.tile_pool(name="headps", bufs=1, space="PSUM") as hps_p,
        ):
            # --- constants into SBUF ---
            # idx first: it gates every gather, so it must not queue behind
            # the 6MB of weight DMAs on the same ring.
            idx_sb = singles.tile([128, G], I32, tag="idx")
            nc.scalar.dma_start(out=idx_sb[:], in_=idx_t[:])
            wsb = {}
            for n in ("wf", "wi", "wh"):
                w = singles.tile([128, EB, u], BF16, tag=n)
                nc.scalar.dma_start(out=w[:], in_=w_t[n][:])
                wsb[n] = w
            bsb = {}
            for n in ("bfv", "biv", "bhv"):
                bb = singles.tile([128, UB], F32, tag=n)
                nc.scalar.dma_start(out=bb[:], in_=b_t[n][:])
                bsb[n] = bb
            w1_sb = singles.tile([128, UB, 64], F32, tag="w1")
            nc.scalar.dma_start(out=w1_sb[:], in_=w1_t[:])
            w2_sb = singles.tile([64, 1], F32, tag="w2")
            nc.scalar.dma_start(out=w2_sb[:], in_=w2_t[:])
            b1_sb = singles.tile([64, 1], F32, tag="b1")
            nc.scalar.dma_start(out=b1_sb[:], in_=b1_t[:])
            b2_sb = singles.tile([1, 1], F32, tag="b2")
            nc.scalar.dma_start(out=b2_sb[:], in_=b2_t[:])

            h_all = singles.tile([128, UB * n_rows], F32, tag="h_all")

            prev_scan = [None] * UB
            for t_i in range(NT):
                row = t_i // tt_per_row
                first = (t_i % tt_per_row) == 0
                last = (t_i % tt_per_row) == tt_per_row - 1

                # xT[p, m, t] = x[t, m*128 + p] in bf16
                xT = xT_p.tile([128, EB, tok_tile], BF16, tag="xT")
                for q in range(QT):
                    g = t_i * QT + q
                    xr = xraw_p.tile([128, e], F32, tag="xr")
                    nc.gpsimd.indirect_dma_start(
                        out=xr[:],
                        out_offset=None,
                        in_=emb_t[:],
                        in_offset=bass.IndirectOffsetOnAxis(
                            ap=idx_sb[:, g : g + 1], axis=0
                        ),
                    )
                    xb = xbf_p.tile([128, e], BF16, tag="xb")
                    nc.scalar.activation(xb[:], xr[:], AF.Copy)
                    nc.sync.dma_start_transpose(
                        out=xT[:, :, ts(q, 128)], in_=xb[:]
                    )

                for ub in range(UB):
                    ps = {}
                    for n in ("wf", "wi", "wh"):
                        p = gps_p.tile([128, tok_tile], F32, tag="gates")
                        for m in range(EB):
                            nc.tensor.matmul(
                                p[:],
                                lhsT=wsb[n][:, m, ts(ub, 128)],
                                rhs=xT[:, m, :],
                                start=(m == 0),
                                stop=(m == EB - 1),
                            )
                        ps[n] = p
                    fsb = sig_p.tile([128, tok_tile], F32, tag="fsb")
                    nc.scalar.activation(
                        fsb[:], ps["wf"][:], AF.Sigmoid,
                        bias=bsb["bfv"][:, ub : ub + 1],
                    )
                    isb = sig_p.tile([128, tok_tile], F32, tag="isb")
                    nc.scalar.activation(
                        isb[:], ps["wi"][:], AF.Sigmoid,
                        bias=bsb["biv"][:, ub : ub + 1],
                    )
                    fn = gw_p.tile([128, tok_tile], F32, tag="fn")
                    nc.vector._custom_dve(
                        FN_OP, out=fn[:], in0=fsb[:], in1=isb[:],
                        s0=RECIP_C0, s1=RECIP_C1,
                    )
                    gg = gw_p.tile([128, tok_tile], F32, tag="gg")
                    nc.vector._custom_dve(
                        GG_OP, out=gg[:], in0=ps["wh"][:], in1=fn[:],
                        s0=bsb["bhv"][:, ub : ub + 1],
                    )
                    sc = scan_p.tile([128, tok_tile], F32, tag="scan")
                    init = (
                        0.0 if first
                        else prev_scan[ub][:, tok_tile - 1 : tok_tile]
                    )
                    nc.vector.tensor_tensor_scan(
                        out=sc[:],
                        data0=fn[:],
                        data1=gg[:],
                        initial=init,
                        op0=ALU.mult,
                        op1=ALU.add,
                    )
                    prev_scan[ub] = sc
                    if last:
                        nc.vector.tensor_copy(
                            out=h_all[:, ub * n_rows + row : ub * n_rows + row + 1],
                            in_=sc[:, tok_tile - 1 : tok_tile],
                        )

            # --- head: z = sigmoid((h@W1 + b1)@W2 + b2) ---
            z1p = hps_p.tile([64, n_rows], F32, tag="z1p")
            for ub in range(UB):
                nc.tensor.matmul(
                    z1p[:],
                    lhsT=w1_sb[:, ub, :],
                    rhs=h_all[:, ts(ub, n_rows)],
                    start=(ub == 0),
                    stop=(ub == UB - 1),
                )
            z1 = singles.tile([64, n_rows], F32, tag="z1")
            nc.vector.tensor_scalar_add(z1[:], z1p[:], b1_sb[:, 0:1])
            z2p = hps_p.tile([1, n_rows], F32, tag="z2p")
            nc.tensor.matmul(z2p[:], lhsT=w2_sb[:], rhs=z1[:], start=True, stop=True)
            outsb = singles.tile([1, n_rows], F32, tag="outsb")
            nc.scalar.activation(outsb[:], z2p[:], AF.Sigmoid, bias=b2_sb[:, 0:1])
            nc.scalar.dma_start(out=out_t[:], in_=outsb[:])

    nc.compile()
    return nc


def make_in_maps(sentence, emb, Wf, bf, Wi, bi, Wh, bh, W1, b1, W2, b2,
                 n_rows=B // N_CORES, n_cores=N_CORES):
    """Shard/repack full inputs into per-core input maps."""
    e = emb.shape[1]
    u = Wf.shape[1]
    EB = e // 128
    UB = u // 128

    def wprep(w):  # [E,U] f32 -> [128, EB, U] bf16 with E = m*128 + p
        return np.ascontiguousarray(
            w.reshape(EB, 128, u).transpose(1, 0, 2)
        ).astype(ml_dtypes.bfloat16)

    def bprep(bv):  # [U] -> [128, UB] with U = ub*128 + p
        return np.ascontiguousarray(bv.reshape(UB, 128).T).astype(np.float32)

    emb_f = np.ascontiguousarray(emb, dtype=np.float32)
    shared = {
        "emb": emb_f,
        "wf": wprep(Wf), "wi": wprep(Wi), "wh": wprep(Wh),
        "bfv": bprep(bf), "biv": bprep(bi), "bhv": bprep(bh),
        "w1": np.ascontiguousarray(
            W1.reshape(UB, 128, 64).transpose(1, 0, 2)
        ).astype(np.float32),
        "w2": np.ascontiguousarray(W2.reshape(64, 1), dtype=np.float32),
        "b1": np.ascontiguousarray(b1.reshape(64, 1), dtype=np.float32),
        "b2": np.ascontiguousarray(b2.reshape(1, 1), dtype=np.float32),
    }
    in_maps = []
    for c in range(n_cores):
        shard = sentence[c * n_rows : (c + 1) * n_rows]  # [n_rows, S]
        idx = np.ascontiguousarray(
            shard.reshape(-1, 128).T.astype(np.int32)
        )  # [128, G], col g = tokens [g*128, (g+1)*128) in row-major order
        in_maps.append({"idx": idx, **shared})
    return in_maps


_NC_CACHE = {}


def kernel(**inputs):
    sentence = np.asarray(inputs["sentence"])
    key = "full"
    if key not in _NC_CACHE:
        _NC_CACHE[key] = build_nc()
    nc = _NC_CACHE[key]
    in_maps = make_in_maps(
        sentence,
        np.asarray(inputs["emb"]), np.asarray(inputs["Wf"]),
        np.asarray(inputs["bf"]), np.asarray(inputs["Wi"]),
        np.asarray(inputs["bi"]), np.asarray(inputs["Wh"]),
        np.asarray(inputs["bh"]), np.asarray(inputs["W1"]),
        np.asarray(inputs["b1"]), np.asarray(inputs["W2"]),
        np.asarray(inputs["b2"]),
    )
    res = run_bass_kernel_spmd(nc, in_maps, core_ids=list(range(N_CORES)))
    outs = [np.asarray(res.results[c]["out"]).reshape(-1) for c in range(N_CORES)]
    return np.concatenate(outs).reshape(B, 1).astype(np.float32)


# revision 12
# speedup vs baseline: 1.0055x; 1.0055x over previous
"""MinRNN Trainium2 Bass kernel.

Problem: minLSTM-style recurrence over sentences.
  x = emb[sentence]                       [B,S,E]
  f = sigmoid(x@Wf + bf); i = sigmoid(x@Wi + bi); h~ = x@Wh + bh
  f_n = f/(f+i); g = (i/(f+i)) * h~
  h_t = f_n_t * h_{t-1} + g_t   (scan over S, only final h needed)
  out = sigmoid((h@W1 + b1)@W2 + b2)      [B,1]

Sharding: data-parallel over batch. 8 cores x 8 rows each. Weights and the
embedding table are replicated; each core gathers its own tokens from the
table with indirect DMA.

Per-core dataflow (all shapes per core; ROWS=8, S=1024, E=U=1024):
  - gather 128-token tiles of emb rows -> SBUF [128 tok, E] f32
  - cast to bf16 (ScalarE), DMA-transpose to xT [128 e, EB, tok] bf16
  - 3 GEMMs on TensorE in bf16 (fp32 PSUM accumulate), N=512 moving dim
  - gate math on ScalarE/VectorE in fp32
  - tensor_tensor_scan on VectorE for the recurrence (fp32 state)
  - tiny head matmuls in fp32, sigmoid, DMA out [1, ROWS]
"""

import sys

if "/opt/trn_rl_repo" not in sys.path:
    sys.path.insert(0, "/opt/trn_rl_repo")

import numpy as np
import ml_dtypes

import concourse.bass as bass
import concourse.bacc as bacc
import concourse.mybir as mybir
from concourse.bass import ts
from concourse.tile import TileContext
from concourse.bass_utils import run_bass_kernel_spmd

N_CORES = 8
B, S, E, U, V = 64, 1024, 1024, 1024, 32000

F32 = mybir.dt.float32
BF16 = mybir.dt.bfloat16
I32 = mybir.dt.int32
AF = mybir.ActivationFunctionType
ALU = mybir.AluOpType


def _register_dve_op(name, spec):
    """Register a custom DVE op at runtime (self-pinning its uops sha)."""
    from concourse import dve_ops
    from concourse.dve_spec import lower, _has_src1
    from concourse.dve_uop import DveOpSpec

    if name in dve_ops.CUSTOM_DVE_SPECS:
        for op in dve_ops.OPS:
            if op.name == name:
                return op
    dve_ops._SUB_OPCODE_FOR_NAME[name] = dve_ops._CUSTOM_DVE_ROW_BASE + len(
        dve_ops.OPS
    )
    shas = {}
    for ver in ("v3", "v4"):
        s = DveOpSpec(
            name=name,
            opcode=dve_ops.get_dve_sub_opcode(name),
            uops=lower(spec, ver=ver),
            rd1_en=_has_src1(spec),
        )
        shas[ver] = s.sha(ver)
    op = dve_ops.DveOp(name, spec, subdim=False, uops_sha=shas)
    dve_ops.OPS.append(op)
    dve_ops.CUSTOM_DVE_SPECS[name] = spec
    return op


def _make_gate_ops():
    """Two fused gate ops:

    MINRNN_FN: fn = f / (f + i) via BITWISE_NOT reciprocal seed + 1 Newton
      step (Chebyshev pair; ~1.7e-3 max rel err on den in (0,2)).
      in0=f, in1=i, s0/s1 = recip constants.
    MINRNN_GG: gg = (h_pre + bh) * (1 - fn).  in0=h_pre(psum), in1=fn, s0=bh.
    """
    import numpy as np
    from concourse.dve_spec import AluOp, Bin, C0, C1, One, Spec, Src0, Src1

    _den = Src0 + Src1
    _nd = Bin(AluOp.BITWISE_NOT, _den, _den)
    _y0 = _nd * C0
    _y1 = _y0 * (C1 - _den * _y0)

    def _ref_fn(in0, in1, c0, c1, c2):
        den = (in0 + in1).astype(np.float32)
        nd = (~den.view(np.int32)).view(np.float32)
        y0 = (nd * np.float32(c0)).astype(np.float32)
        y1 = (y0 * (np.float32(c1) - den * y0)).astype(np.float32)
        return (in0 * y1).astype(np.float32)

    fn_op = _register_dve_op(
        "MINRNN_FN", Spec(body=Src0 * _y1, reference=_ref_fn)
    )

    def _ref_gg(in0, in1, c0, c1, c2):
        c0 = np.asarray(c0, np.float32)
        return ((in0 + c0) * (np.float32(1.0) - in1)).astype(np.float32)

    gg_op = _register_dve_op(
        "MINRNN_GG",
        Spec(body=(Src0 + C0) * (One - Src1), reference=_ref_gg),
    )
    return fn_op, gg_op


RECIP_C0 = -0.23549792
RECIP_C1 = 2.0017324


def build_nc(n_rows=B // N_CORES, s=S, e=E, u=U, v=V, tok_tile=512):
    """Build the single-core program (SPMD: same program on all cores)."""
    assert s % tok_tile == 0 and tok_tile % 128 == 0
    toks = n_rows * s            # tokens per core
    G = toks // 128              # number of 128-row gathers
    NT = toks // tok_tile        # token tiles
    QT = tok_tile // 128         # gathers per token tile
    EB = e // 128                # contraction blocks
    UB = u // 128                # output-unit blocks
    tt_per_row = s // tok_tile

    nc = bacc.Bacc("TRN2", target_bir_lowering=False)
    FN_OP, GG_OP = _make_gate_ops()

    idx_t = nc.dram_tensor("idx", [128, G], I32, kind="ExternalInput")
    emb_t = nc.dram_tensor("emb", [v, e], F32, kind="ExternalInput")
    w_t = {
        n: nc.dram_tensor(n, [128, EB, u], BF16, kind="ExternalInput")
        for n in ("wf", "wi", "wh")
    }
    b_t = {
        n: nc.dram_tensor(n, [128, UB], F32, kind="ExternalInput")
        for n in ("bfv", "biv", "bhv")
    }
    w1_t = nc.dram_tensor("w1", [128, UB, 64], F32, kind="ExternalInput")
    w2_t = nc.dram_tensor("w2", [64, 1], F32, kind="ExternalInput")
    b1_t = nc.dram_tensor("b1", [64, 1], F32, kind="ExternalInput")
    b2_t = nc.dram_tensor("b2", [1, 1], F32, kind="ExternalInput")
    out_t = nc.dram_tensor("out", [1, n_rows], F32, kind="ExternalOutput")

    with TileContext(nc) as tc:
        with (
            tc.tile_pool(name="singles", bufs=1) as singles,
            tc.tile_pool(name="xraw", bufs=4) as xraw_p,
            tc.tile_pool(name="xbf", bufs=4) as xbf_p,
            tc.tile_pool(name="xT", bufs=3) as xT_p,
            tc.tile_pool(name="sig", bufs=4) as sig_p,
            tc.tile_pool(name="gw", bufs=3) as gw_p,
            tc.tile_pool(name="scan", bufs=2 * UB) as scan_p,
            tc.tile_pool(name="gates", bufs=7, space="PSUM") as gps_p,
            tc.tile_pool(name="headps", bufs=1, space="PSUM") as hps_p,
        ):
            # --- constants into SBUF ---
            # idx first: it gates every gather, so it must not queue behind
            # the 6MB of weight DMAs on the same ring.
            idx_sb = singles.tile([128, G], I32, tag="idx")
            nc.scalar.dma_start(out=idx_sb[:], in_=idx_t[:])
            wsb = {}
            for n in ("wf", "wi", "wh"):
                w = singles.tile([128, EB, u], BF16, tag=n)
                nc.scalar.dma_start(out=w[:], in_=w_t[n][:])
                wsb[n] = w
            bsb = {}
            for n in ("bfv", "biv", "bhv"):
                bb = singles.tile([128, UB], F32, tag=n)
                nc.scalar.dma_start(out=bb[:], in_=b_t[n][:])
                bsb[n] = bb
            w1_sb = singles.tile([128, UB, 64], F32, tag="w1")
            nc.scalar.dma_start(out=w1_sb[:], in_=w1_t[:])
            w2_sb = singles.tile([64, 1], F32, tag="w2")
            nc.scalar.dma_start(out=w2_sb[:], in_=w2_t[:])
            b1_sb = singles.tile([64, 1], F32, tag="b1")
            nc.scalar.dma_start(out=b1_sb[:], in_=b1_t[:])
            b2_sb = singles.tile([1, 1], F32, tag="b2")
            nc.scalar.dma_start(out=b2_sb[:], in_=b2_t[:])

            h_all = singles.tile([128, UB * n_rows], F32, tag="h_all")

            prev_scan = [None] * UB
            for t_i in range(NT):
                row = t_i // tt_per_row
                first = (t_i % tt_per_row) == 0
                last = (t_i % tt_per_row) == tt_per_row - 1

                # xT[p, m, t] = x[t, m*128 + p] in bf16
                xT = xT_p.tile([128, EB, tok_tile], BF16, tag="xT")
                for q in range(QT):
                    g = t_i * QT + q
                    xr = xraw_p.tile([128, e], F32, tag="xr")
                    nc.gpsimd.indirect_dma_start(
                        out=xr[:],
                        out_offset=None,
                        in_=emb_t[:],
                        in_offset=bass.IndirectOffsetOnAxis(
                            ap=idx_sb[:, g : g + 1], axis=0
                        ),
                    )
                    xb = xbf_p.tile([128, e], BF16, tag="xb")
                    nc.scalar.activation(xb[:], xr[:], AF.Copy)
                    nc.sync.dma_start_transpose(
                        out=xT[:, :, ts(q, 128)], in_=xb[:]
                    )

                for ub in range(UB):
                    ps = {}
                    for n in ("wf", "wi", "wh"):
                        p = gps_p.tile([128, tok_tile], F32, tag="gates")
                        for m in range(EB):
                            nc.tensor.matmul(
                                p[:],
                                lhsT=wsb[n][:, m, ts(ub, 128)],
                                rhs=xT[:, m, :],
                                start=(m == 0),
                                stop=(m == EB - 1),
                            )
                        ps[n] = p
                    fsb = sig_p.tile([128, tok_tile], F32, tag="fsb")
                    nc.scalar.activation(
                        fsb[:], ps["wf"][:], AF.Sigmoid,
                        bias=bsb["bfv"][:, ub : ub + 1],
                    )
                    isb = sig_p.tile([128, tok_tile], F32, tag="isb")
                    nc.scalar.activation(
                        isb[:], ps["wi"][:], AF.Sigmoid,
                        bias=bsb["biv"][:, ub : ub + 1],
                    )
                    fn = gw_p.tile([128, tok_tile], F32, tag="fn")
                    nc.vector._custom_dve(
                        FN_OP, out=fn[:], in0=fsb[:], in1=isb[:],
                        s0=RECIP_C0, s1=RECIP_C1,
                    )
                    gg = gw_p.tile([128, tok_tile], F32, tag="gg")
                    nc.vector._custom_dve(
                        GG_OP, out=gg[:], in0=ps["wh"][:], in1=fn[:],
                        s0=bsb["bhv"][:, ub : ub + 1],
                    )
                    sc = scan_p.tile([128, tok_tile], F32, tag="scan")
                    init = (
                        0.0 if first
                        else prev_scan[ub][:, tok_tile - 1 : tok_tile]
                    )
                    nc.vector.tensor_tensor_scan(
                        out=sc[:],
                        data0=fn[:],
                        data1=gg[:],
                        initial=init,
                        op0=ALU.mult,
                        op1=ALU.add,
                    )
                    prev_scan[ub] = sc
                    if last:
                        nc.vector.tensor_copy(
                            out=h_all[:, ub * n_rows + row : ub * n_rows + row + 1],
                            in_=sc[:, tok_tile - 1 : tok_tile],
                        )

            # --- head: z = sigmoid((h@W1 + b1)@W2 + b2) ---
            z1p = hps_p.tile([64, n_rows], F32, tag="hps")
            for ub in range(UB):
                nc.tensor.matmul(
                    z1p[:],
                    lhsT=w1_sb[:, ub, :],
                    rhs=h_all[:, ts(ub, n_rows)],
                    start=(ub == 0),
                    stop=(ub == UB - 1),
                )
            z1 = singles.tile([64, n_rows], F32, tag="z1")
            nc.vector.tensor_scalar_add(z1[:], z1p[:], b1_sb[:, 0:1])
            z2p = hps_p.tile([1, n_rows], F32, tag="hps")
            nc.tensor.matmul(z2p[:], lhsT=w2_sb[:], rhs=z1[:], start=True, stop=True)
            outsb = singles.tile([1, n_rows], F32, tag="outsb")
            nc.scalar.activation(outsb[:], z2p[:], AF.Sigmoid, bias=b2_sb[:, 0:1])
            nc.scalar.dma_start(out=out_t[:], in_=outsb[:])

    nc.compile()
    return nc


def make_in_maps(sentence, emb, Wf, bf, Wi, bi, Wh, bh, W1, b1, W2, b2,
                 n_rows=B // N_CORES, n_cores=N_CORES):
    """Shard/repack full inputs into per-core input maps."""
    e = emb.shape[1]
    u = Wf.shape[1]
    EB = e // 128
    UB = u // 128

    def wprep(w):  # [E,U] f32 -> [128, EB, U] bf16 with E = m*128 + p
        return np.ascontiguousarray(
            w.reshape(EB, 128, u).transpose(1, 0, 2)
        ).astype(ml_dtypes.bfloat16)

    def bprep(bv):  # [U] -> [128, UB] with U = ub*128 + p
        return np.ascontiguousarray(bv.reshape(UB, 128).T).astype(np.float32)

    emb_f = np.ascontiguousarray(emb, dtype=np.float32)
    shared = {
        "emb": emb_f,
        "wf": wprep(Wf), "wi": wprep(Wi), "wh": wprep(Wh),
        "bfv": bprep(bf), "biv": bprep(bi), "bhv": bprep(bh),
        "w1": np.ascontiguousarray(
            W1.reshape(UB, 128, 64).transpose(1, 0, 2)
        ).astype(np.float32),
        "w2": np.ascontiguousarray(W2.reshape(64, 1), dtype=np.float32),
        "b1": np.ascontiguousarray(b1.reshape(64, 1), dtype=np.float32),
        "b2": np.ascontiguousarray(b2.reshape(1, 1), dtype=np.float32),
    }
    in_maps = []
    for c in range(n_cores):
        shard = sentence[c * n_rows : (c + 1) * n_rows]  # [n_rows, S]
        idx = np.ascontiguousarray(
            shard.reshape(-1, 128).T.astype(np.int32)
        )  # [128, G], col g = tokens [g*128, (g+1)*128) in row-major order
        in_maps.append({"idx": idx, **shared})
    return in_maps


_NC_CACHE = {}


def kernel(**inputs):
    sentence = np.asarray(inputs["sentence"])
    key = "full"
    if key not in _NC_CACHE:
        _NC_CACHE[key] = build_nc()
    nc = _NC_CACHE[key]
    in_maps = make_in_maps(
        sentence,
        np.asarray(inputs["emb"]), np.asarray(inputs["Wf"]),
        np.asarray(inputs["bf"]), np.asarray(inputs["Wi"]),
        np.asarray(inputs["bi"]), np.asarray(inputs["Wh"]),
        np.asarray(inputs["bh"]), np.asarray(inputs["W1"]),
        np.asarray(inputs["b1"]), np.asarray(inputs["W2"]),
        np.asarray(inputs["b2"]),
    )
    res = run_bass_kernel_spmd(nc, in_maps, core_ids=list(range(N_CORES)))
    outs = [np.asarray(res.results[c]["out"]).reshape(-1) for c in range(N_CORES)]
    return np.concatenate(outs).reshape(B, 1).astype(np.float32)


# revision 13
# speedup vs baseline: 1.0169x; 1.0113x over previous
"""MinRNN Trainium2 Bass kernel.

Problem: minLSTM-style recurrence over sentences.
  x = emb[sentence]                       [B,S,E]
  f = sigmoid(x@Wf + bf); i = sigmoid(x@Wi + bi); h~ = x@Wh + bh
  f_n = f/(f+i); g = (i/(f+i)) * h~
  h_t = f_n_t * h_{t-1} + g_t   (scan over S, only final h needed)
  out = sigmoid((h@W1 + b1)@W2 + b2)      [B,1]

Sharding: data-parallel over batch. 8 cores x 8 rows each. Weights and the
embedding table are replicated; each core gathers its own tokens from the
table with indirect DMA.

Per-core dataflow (all shapes per core; ROWS=8, S=1024, E=U=1024):
  - gather 128-token tiles of emb rows -> SBUF [128 tok, E] f32
  - cast to bf16 (ScalarE), DMA-transpose to xT [128 e, EB, tok] bf16
  - 3 GEMMs on TensorE in bf16 (fp32 PSUM accumulate), N=512 moving dim
  - gate math on ScalarE/VectorE in fp32
  - tensor_tensor_scan on VectorE for the recurrence (fp32 state)
  - tiny head matmuls in fp32, sigmoid, DMA out [1, ROWS]
"""

import sys

if "/opt/trn_rl_repo" not in sys.path:
    sys.path.insert(0, "/opt/trn_rl_repo")

import numpy as np
import ml_dtypes

import concourse.bass as bass
import concourse.bacc as bacc
import concourse.mybir as mybir
from concourse.bass import ts
from concourse.tile import TileContext
from concourse.bass_utils import run_bass_kernel_spmd

N_CORES = 8
B, S, E, U, V = 64, 1024, 1024, 1024, 32000

F32 = mybir.dt.float32
BF16 = mybir.dt.bfloat16
I32 = mybir.dt.int32
AF = mybir.ActivationFunctionType
ALU = mybir.AluOpType


def _register_dve_op(name, spec):
    """Register a custom DVE op at runtime (self-pinning its uops sha)."""
    from concourse import dve_ops
    from concourse.dve_spec import lower, _has_src1
    from concourse.dve_uop import DveOpSpec

    if name in dve_ops.CUSTOM_DVE_SPECS:
        for op in dve_ops.OPS:
            if op.name == name:
                return op
    dve_ops._SUB_OPCODE_FOR_NAME[name] = dve_ops._CUSTOM_DVE_ROW_BASE + len(
        dve_ops.OPS
    )
    shas = {}
    for ver in ("v3", "v4"):
        s = DveOpSpec(
            name=name,
            opcode=dve_ops.get_dve_sub_opcode(name),
            uops=lower(spec, ver=ver),
            rd1_en=_has_src1(spec),
        )
        shas[ver] = s.sha(ver)
    op = dve_ops.DveOp(name, spec, subdim=False, uops_sha=shas)
    dve_ops.OPS.append(op)
    dve_ops.CUSTOM_DVE_SPECS[name] = spec
    return op


def _make_gate_ops():
    """Two fused gate ops:

    MINRNN_FN: fn = f / (f + i) via BITWISE_NOT reciprocal seed + 1 Newton
      step (Chebyshev pair; ~1.7e-3 max rel err on den in (0,2)).
      in0=f, in1=i, s0/s1 = recip constants.
    MINRNN_GG: gg = (h_pre + bh) * (1 - fn).  in0=h_pre(psum), in1=fn, s0=bh.
    """
    import numpy as np
    from concourse.dve_spec import AluOp, Bin, C0, C1, One, Spec, Src0, Src1

    _den = Src0 + Src1
    _nd = Bin(AluOp.BITWISE_NOT, _den, _den)
    _y0 = _nd * C0
    _y1 = _y0 * (C1 - _den * _y0)

    def _ref_fn(in0, in1, c0, c1, c2):
        den = (in0 + in1).astype(np.float32)
        nd = (~den.view(np.int32)).view(np.float32)
        y0 = (nd * np.float32(c0)).astype(np.float32)
        y1 = (y0 * (np.float32(c1) - den * y0)).astype(np.float32)
        return (in0 * y1).astype(np.float32)

    fn_op = _register_dve_op(
        "MINRNN_FN", Spec(body=Src0 * _y1, reference=_ref_fn)
    )

    def _ref_gg(in0, in1, c0, c1, c2):
        c0 = np.asarray(c0, np.float32)
        return ((in0 + c0) * (np.float32(1.0) - in1)).astype(np.float32)

    gg_op = _register_dve_op(
        "MINRNN_GG",
        Spec(body=(Src0 + C0) * (One - Src1), reference=_ref_gg),
    )
    return fn_op, gg_op


RECIP_C0 = -0.23549792
RECIP_C1 = 2.0017324


def build_nc(n_rows=B // N_CORES, s=S, e=E, u=U, v=V, tok_tile=512):
    """Build the single-core program (SPMD: same program on all cores)."""
    assert s % tok_tile == 0 and tok_tile % 128 == 0
    toks = n_rows * s            # tokens per core
    G = toks // 128              # number of 128-row gathers
    NT = toks // tok_tile        # token tiles
    QT = tok_tile // 128         # gathers per token tile
    EB = e // 128                # contraction blocks
    UB = u // 128                # output-unit blocks
    tt_per_row = s // tok_tile

    nc = bacc.Bacc("TRN2", target_bir_lowering=False)
    FN_OP, GG_OP = _make_gate_ops()

    idx_t = nc.dram_tensor("idx", [128, G], I32, kind="ExternalInput")
    emb_t = nc.dram_tensor("emb", [v, e], BF16, kind="ExternalInput")
    w_t = {
        n: nc.dram_tensor(n, [128, EB, u], BF16, kind="ExternalInput")
        for n in ("wf", "wi", "wh")
    }
    b_t = {
        n: nc.dram_tensor(n, [128, UB], F32, kind="ExternalInput")
        for n in ("bfv", "biv", "bhv")
    }
    w1_t = nc.dram_tensor("w1", [128, UB, 64], F32, kind="ExternalInput")
    w2_t = nc.dram_tensor("w2", [64, 1], F32, kind="ExternalInput")
    b1_t = nc.dram_tensor("b1", [64, 1], F32, kind="ExternalInput")
    b2_t = nc.dram_tensor("b2", [1, 1], F32, kind="ExternalInput")
    out_t = nc.dram_tensor("out", [1, n_rows], F32, kind="ExternalOutput")

    with TileContext(nc) as tc:
        with (
            tc.tile_pool(name="singles", bufs=1) as singles,
            tc.tile_pool(name="xraw", bufs=6) as xraw_p,
            tc.tile_pool(name="xT", bufs=3) as xT_p,
            tc.tile_pool(name="sig", bufs=4) as sig_p,
            tc.tile_pool(name="gw", bufs=3) as gw_p,
            tc.tile_pool(name="scan", bufs=2 * UB) as scan_p,
            tc.tile_pool(name="gates", bufs=7, space="PSUM") as gps_p,
            tc.tile_pool(name="headps", bufs=1, space="PSUM") as hps_p,
        ):
            # --- constants into SBUF ---
            # idx first: it gates every gather, so it must not queue behind
            # the 6MB of weight DMAs on the same ring.
            idx_sb = singles.tile([128, G], I32, tag="idx")
            nc.scalar.dma_start(out=idx_sb[:], in_=idx_t[:])
            wsb = {}
            for n in ("wf", "wi", "wh"):
                w = singles.tile([128, EB, u], BF16, tag=n)
                nc.scalar.dma_start(out=w[:], in_=w_t[n][:])
                wsb[n] = w
            bsb = {}
            for n in ("bfv", "biv", "bhv"):
                bb = singles.tile([128, UB], F32, tag=n)
                nc.scalar.dma_start(out=bb[:], in_=b_t[n][:])
                bsb[n] = bb
            w1_sb = singles.tile([128, UB, 64], F32, tag="w1")
            nc.scalar.dma_start(out=w1_sb[:], in_=w1_t[:])
            w2_sb = singles.tile([64, 1], F32, tag="w2")
            nc.scalar.dma_start(out=w2_sb[:], in_=w2_t[:])
            b1_sb = singles.tile([64, 1], F32, tag="b1")
            nc.scalar.dma_start(out=b1_sb[:], in_=b1_t[:])
            b2_sb = singles.tile([1, 1], F32, tag="b2")
            nc.scalar.dma_start(out=b2_sb[:], in_=b2_t[:])

            h_all = singles.tile([128, UB * n_rows], F32, tag="h_all")

            prev_scan = [None] * UB
            for t_i in range(NT):
                row = t_i // tt_per_row
                first = (t_i % tt_per_row) == 0
                last = (t_i % tt_per_row) == tt_per_row - 1

                # xT[p, m, t] = x[t, m*128 + p] in bf16
                xT = xT_p.tile([128, EB, tok_tile], BF16, tag="xT")
                for q in range(QT):
                    g = t_i * QT + q
                    xr = xraw_p.tile([128, e], BF16, tag="xr")
                    nc.gpsimd.indirect_dma_start(
                        out=xr[:],
                        out_offset=None,
                        in_=emb_t[:],
                        in_offset=bass.IndirectOffsetOnAxis(
                            ap=idx_sb[:, g : g + 1], axis=0
                        ),
                    )
                    nc.sync.dma_start_transpose(
                        out=xT[:, :, ts(q, 128)], in_=xr[:]
                    )

                for ub in range(UB):
                    ps = {}
                    for n in ("wf", "wi", "wh"):
                        p = gps_p.tile([128, tok_tile], F32, tag="gates")
                        for m in range(EB):
                            nc.tensor.matmul(
                                p[:],
                                lhsT=wsb[n][:, m, ts(ub, 128)],
                                rhs=xT[:, m, :],
                                start=(m == 0),
                                stop=(m == EB - 1),
                            )
                        ps[n] = p
                    fsb = sig_p.tile([128, tok_tile], F32, tag="fsb")
                    nc.scalar.activation(
                        fsb[:], ps["wf"][:], AF.Sigmoid,
                        bias=bsb["bfv"][:, ub : ub + 1],
                    )
                    isb = sig_p.tile([128, tok_tile], F32, tag="isb")
                    nc.scalar.activation(
                        isb[:], ps["wi"][:], AF.Sigmoid,
                        bias=bsb["biv"][:, ub : ub + 1],
                    )
                    fn = gw_p.tile([128, tok_tile], F32, tag="fn")
                    nc.vector._custom_dve(
                        FN_OP, out=fn[:], in0=fsb[:], in1=isb[:],
                        s0=RECIP_C0, s1=RECIP_C1,
                    )
                    gg = gw_p.tile([128, tok_tile], F32, tag="gg")
                    nc.vector._custom_dve(
                        GG_OP, out=gg[:], in0=ps["wh"][:], in1=fn[:],
                        s0=bsb["bhv"][:, ub : ub + 1],
                    )
                    sc = scan_p.tile([128, tok_tile], F32, tag="scan")
                    init = (
                        0.0 if first
                        else prev_scan[ub][:, tok_tile - 1 : tok_tile]
                    )
                    nc.vector.tensor_tensor_scan(
                        out=sc[:],
                        data0=fn[:],
                        data1=gg[:],
                        initial=init,
                        op0=ALU.mult,
                        op1=ALU.add,
                    )
                    prev_scan[ub] = sc
                    if last:
                        nc.vector.tensor_copy(
                            out=h_all[:, ub * n_rows + row : ub * n_rows + row + 1],
                            in_=sc[:, tok_tile - 1 : tok_tile],
                        )

            # --- head: z = sigmoid((h@W1 + b1)@W2 + b2) ---
            z1p = hps_p.tile([64, n_rows], F32, tag="hps")
            for ub in range(UB):
                nc.tensor.matmul(
                    z1p[:],
                    lhsT=w1_sb[:, ub, :],
                    rhs=h_all[:, ts(ub, n_rows)],
                    start=(ub == 0),
                    stop=(ub == UB - 1),
                )
            z1 = singles.tile([64, n_rows], F32, tag="z1")
            nc.vector.tensor_scalar_add(z1[:], z1p[:], b1_sb[:, 0:1])
            z2p = hps_p.tile([1, n_rows], F32, tag="hps")
            nc.tensor.matmul(z2p[:], lhsT=w2_sb[:], rhs=z1[:], start=True, stop=True)
            outsb = singles.tile([1, n_rows], F32, tag="outsb")
            nc.scalar.activation(outsb[:], z2p[:], AF.Sigmoid, bias=b2_sb[:, 0:1])
            nc.scalar.dma_start(out=out_t[:], in_=outsb[:])

    nc.compile()
    return nc


def make_in_maps(sentence, emb, Wf, bf, Wi, bi, Wh, bh, W1, b1, W2, b2,
                 n_rows=B // N_CORES, n_cores=N_CORES):
    """Shard/repack full inputs into per-core input maps."""
    e = emb.shape[1]
    u = Wf.shape[1]
    EB = e // 128
    UB = u // 128

    def wprep(w):  # [E,U] f32 -> [128, EB, U] bf16 with E = m*128 + p
        return np.ascontiguousarray(
            w.reshape(EB, 128, u).transpose(1, 0, 2)
        ).astype(ml_dtypes.bfloat16)

    def bprep(bv):  # [U] -> [128, UB] with U = ub*128 + p
        return np.ascontiguousarray(bv.reshape(UB, 128).T).astype(np.float32)

    emb_f = np.ascontiguousarray(emb, dtype=np.float32).astype(ml_dtypes.bfloat16)
    shared = {
        "emb": emb_f,
        "wf": wprep(Wf), "wi": wprep(Wi), "wh": wprep(Wh),
        "bfv": bprep(bf), "biv": bprep(bi), "bhv": bprep(bh),
        "w1": np.ascontiguousarray(
            W1.reshape(UB, 128, 64).transpose(1, 0, 2)
        ).astype(np.float32),
        "w2": np.ascontiguousarray(W2.reshape(64, 1), dtype=np.float32),
        "b1": np.ascontiguousarray(b1.reshape(64, 1), dtype=np.float32),
        "b2": np.ascontiguousarray(b2.reshape(1, 1), dtype=np.float32),
    }
    in_maps = []
    for c in range(n_cores):
        shard = sentence[c * n_rows : (c + 1) * n_rows]  # [n_rows, S]
        idx = np.ascontiguousarray(
            shard.reshape(-1, 128).T.astype(np.int32)
        )  # [128, G], col g = tokens [g*128, (g+1)*128) in row-major order
        in_maps.append({"idx": idx, **shared})
    return in_maps


_NC_CACHE = {}


def kernel(**inputs):
    sentence = np.asarray(inputs["sentence"])
    key = "full"
    if key not in _NC_CACHE:
        _NC_CACHE[key] = build_nc()
    nc = _NC_CACHE[key]
    in_maps = make_in_maps(
        sentence,
        np.asarray(inputs["emb"]), np.asarray(inputs["Wf"]),
        np.asarray(inputs["bf"]), np.asarray(inputs["Wi"]),
        np.asarray(inputs["bi"]), np.asarray(inputs["Wh"]),
        np.asarray(inputs["bh"]), np.asarray(inputs["W1"]),
        np.asarray(inputs["b1"]), np.asarray(inputs["W2"]),
        np.asarray(inputs["b2"]),
    )
    res = run_bass_kernel_spmd(nc, in_maps, core_ids=list(range(N_CORES)))
    outs = [np.asarray(res.results[c]["out"]).reshape(-1) for c in range(N_CORES)]
    return np.concatenate(outs).reshape(B, 1).astype(np.float32)
